# revision 21
# baseline (speedup 1.0000x reference)
"""DepthSelfAttention Trainium2 kernel (8-core data-parallel SPMD).

Contract: kernel(**inputs) takes FULL unsharded numpy inputs, returns the
FULL [4, 2048, 2048] fp32 output. Everything is hardcoded for the shapes in
the problem spec.
"""

import sys

sys.path.insert(0, "/opt/trn_rl_repo")

from contextlib import ExitStack

import ml_dtypes
import numpy as np

import concourse.bass as bass
import concourse.tile as tile
from concourse import bacc, mybir

F16 = mybir.dt.float16
F32 = mybir.dt.float32
NPF16 = ml_dtypes.float16 if not hasattr(np, "float16") else np.float16

DIM = 2048
NH = 16
NKV = 4
HD = 128
DEP = 8  # 7 history + current
NCORES = 8
EPS = 1.1920929e-07
SCALE = 1.0 / float(np.sqrt(HD))
ROPE_BASE = 10000.0
MAX_DEPTH = 16
DC = DIM // 128  # 16 contraction chunks
EXP_SHIFT = -4.0  # exp(s + EXP_SHIFT): softmax-invariant, avoids fp16 overflow

LAST_RESULTS = None
_PROGRAM_CACHE = {}


# ---------------------------------------------------------------- host tables
def _rope_tables():
    inv_freq = 1.0 / ROPE_BASE ** (
        np.arange(0, HD, 2, dtype=np.float32) / HD
    )  # [64]
    pos = np.arange(MAX_DEPTH, dtype=np.float32)
    rpos = np.arange(MAX_DEPTH - 1, -1, -1, dtype=np.float32)
    fw = np.outer(pos, inv_freq)
    rv = np.outer(rpos, inv_freq)
    return np.cos(fw), np.sin(fw), np.cos(rv), np.sin(rv)  # each [16, 64]


def _rot_matrix(dcos, dsin, rcos, rsin):
    """R s.t. rope(v) = R @ v for one 128-dim head, given table rows [64]."""
    h = HD // 2
    R = np.zeros((HD, HD), np.float32)
    for m in range(h):
        R[m, m] = dcos[m]
        R[m, m + h] = dsin[m]
    for j in range(h):
        R[h + j, j] = -rsin[j]
        R[h + j, h + j] = rcos[j]
    return R


def _host_constants(q_gain):
    DCOS, DSIN, RCOS, RSIN = _rope_tables()
    d = DEP - 1  # query position index (depth-1 = 7)
    rq = _rot_matrix(DCOS[d], DSIN[d], RCOS[d], RSIN[d]).T  # lhsT = R^T
    rk = np.stack(
        [_rot_matrix(DCOS[j], DSIN[j], RCOS[j], RSIN[j]).T for j in range(DEP)]
    )  # [8, 128, 128] (each R^T)

    # rk layout for SBUF [128, 8, 128]
    rk_sb = np.ascontiguousarray(rk.transpose(1, 0, 2))

    ohq = np.zeros((128, NH, 128), np.float32)
    for hh in range(NH):
        ohq[:, hh, 8 * hh : 8 * hh + 8] = 1.0

    ohk = np.zeros((128, NKV * DEP, 128), np.float32)
    for g in range(NKV):
        for dep in range(DEP):
            for i in range(4):
                c = (4 * g + i) * 8 + dep
                ohk[:, g * 8 + dep, c] = 1.0

    ohsc = np.zeros((128, 256), np.float32)
    ohsc[:, 128] = 1.0

    bd = np.zeros((128, 128), np.float32)
    for c1 in range(128):
        for c2 in range(128):
            if c1 // 8 == c2 // 8:
                bd[c1, c2] = 1.0

    ident = np.eye(128, dtype=np.float32)

    gain = np.asarray(q_gain, np.float64)  # [16]
    gc = gain[np.arange(128) // 8]  # per combo
    kscale = (1.0 / gc**2).astype(np.float32).reshape(128, 1)
    kbias = (128.0 * EPS / gc**2).astype(np.float32).reshape(128, 1)
    qbias = np.full((128, 1), EPS, np.float32)
    ebias = np.full((128, 1), EXP_SHIFT, np.float32)

    f16 = np.float16
    return {
        "rq": rq.astype(f16),
        "rk": rk_sb.astype(f16),
        "ohq": ohq.astype(f16),
        "ohk": ohk.astype(f16),
        "ohsc": ohsc.astype(f16),
        "bd": bd.astype(f16),
        "ident": ident.astype(f16),
        "cols": np.concatenate([kscale, kbias, qbias, ebias], axis=1),
    }


def _prep_weights(Wq, Wk, Wv, Wproj):
    f16 = np.float16

    def t_part(w):  # [out, din] -> [128, din//128, out]
        wt = np.ascontiguousarray(w.T)  # [din, out]
        no = wt.shape[1]
        return np.ascontiguousarray(
            wt.reshape(wt.shape[0] // 128, 128, no).transpose(1, 0, 2)
        ).astype(f16)

    return {
        "wq": t_part(Wq),  # [128, 16, 2048]
        "wk": t_part(Wk),  # [128, 16, 512]
        "wv": t_part(Wv),  # [128, 16, 512]
        "wp": t_part(Wproj),  # [128, 16, 2048]
    }


def _prep_kvt(x, depth_history):
    """-> kvt [8, 128, 16, N] fp16: kvt[dep, p, c, n] = kv[n, dep, 128*c+p]."""
    B, S, D = x.shape
    N = B * S
    xf = x.reshape(N, 1, D)
    dh = depth_history.reshape(N, DEP - 1, D)
    kv = np.concatenate([dh, xf], axis=1).astype(np.float16)  # [N, 8, D]
    kvt = kv.transpose(1, 2, 0).reshape(DEP, DC, 128, N).transpose(0, 2, 1, 3)
    return kvt  # strided view; sliced/contiguized per core


# ---------------------------------------------------------------- device code
def build_program(npc):
    """One-core SPMD program for npc positions (blk = min(512, npc))."""
    blk = min(512, npc)
    nblk = npc // blk
    nsubs = blk // 128
    assert npc % blk == 0 and blk % 128 == 0

    nc = bacc.Bacc()
    kvt_d = nc.declare_dram_parameter("kvt", [DEP, 128, DC, npc], F16, isOutput=False)
    wq_d = nc.declare_dram_parameter("wq", [128, DC, DIM], F16, isOutput=False)
    wk_d = nc.declare_dram_parameter("wk", [128, DC, 512], F16, isOutput=False)
    wv_d = nc.declare_dram_parameter("wv", [128, DC, 512], F16, isOutput=False)
    wp_d = nc.declare_dram_parameter("wp", [128, DC, DIM], F16, isOutput=False)
    rq_d = nc.declare_dram_parameter("rq", [128, 128], F16, isOutput=False)
    rk_d = nc.declare_dram_parameter("rk", [128, DEP, 128], F16, isOutput=False)
    ohq_d = nc.declare_dram_parameter("ohq", [128, NH, 128], F16, isOutput=False)
    ohk_d = nc.declare_dram_parameter("ohk", [128, NKV * DEP, 128], F16, isOutput=False)
    ohsc_d = nc.declare_dram_parameter("ohsc", [128, 256], F16, isOutput=False)
    bd_d = nc.declare_dram_parameter("bd", [128, 128], F16, isOutput=False)
    id_d = nc.declare_dram_parameter("ident", [128, 128], F16, isOutput=False)
    cols_d = nc.declare_dram_parameter("cols", [128, 4], F32, isOutput=False)
    out_d = nc.declare_dram_parameter("out", [npc, DIM], F32, isOutput=True)
    # DRAM scratch for yT between attention and output projection
    yt_d = nc.dram_tensor("yt_scratch", [nblk, nsubs, 128, DC, 128], F16)

    AF = mybir.ActivationFunctionType

    with tile.TileContext(nc) as tc, ExitStack() as top:
        const = top.enter_context(tc.tile_pool(name="const", bufs=1))

        def load_const(dram, shape, name):
            t = const.tile(shape, dram.dtype, name=name)
            nc.sync.dma_start(t[:], dram[:])
            return t

        rq_sb = load_const(rq_d, [128, 128], "rq_sb")
        rk_sb = load_const(rk_d, [128, DEP, 128], "rk_sb")
        ohq_sb = load_const(ohq_d, [128, NH, 128], "ohq_sb")
        ohk_sb = load_const(ohk_d, [128, NKV * DEP, 128], "ohk_sb")
        ohsc_sb = load_const(ohsc_d, [128, 256], "ohsc_sb")
        bd_sb = load_const(bd_d, [128, 128], "bd_sb")
        id_sb = load_const(id_d, [128, 128], "id_sb")
        cols_sb = load_const(cols_d, [128, 4], "cols_sb")
        ksc_sb = cols_sb[:, 0:1]
        kbi_sb = cols_sb[:, 1:2]
        qbi_sb = cols_sb[:, 2:3]
        ebi_sb = cols_sb[:, 3:4]

        # long-lived per-block tensors
        qrotp = top.enter_context(tc.tile_pool(name="qrotp", bufs=2))
        sqqp = top.enter_context(tc.tile_pool(name="sqqp", bufs=2))
        qrot_b, sqq_b = [], []

        # ---------------- Phase Q ----------------
        with (
            tc.tile_pool(name="wqp", bufs=1) as wqp,
            tc.tile_pool(name="xtp", bufs=2) as xtp,
            tc.tile_pool(name="qps", bufs=2, space="PSUM") as qps,
            tc.tile_pool(name="qsb", bufs=3) as qsb,
        ):
            wq_sb = wqp.tile([128, DC, DIM], F16, name="wq_sb")
            nc.sync.dma_start(wq_sb[:], wq_d[:])
            for b in range(nblk):
                xt = xtp.tile([128, DC, blk], F16, name="xt", tag="xt")
                nc.sync.dma_start(
                    xt[:], kvt_d[DEP - 1, :, :, b * blk : (b + 1) * blk]
                )
                sq_ps = qps.tile([128, blk], F32, name="sqq_ps", tag="sumq")
                qro = qrotp.tile([128, NH, blk], F16, name="qrot", tag="qrot")
                for oc in range(NH):
                    qp = qps.tile([128, blk], F32, name="q_ps", tag="qps")
                    for c in range(DC):
                        nc.tensor.matmul(
                            qp[:],
                            wq_sb[:, c, oc * 128 : (oc + 1) * 128],
                            xt[:, c, :],
                            start=(c == 0),
                            stop=(c == DC - 1),
                        )
                    qraw = qsb.tile([128, blk], F16, name="qraw", tag="qraw")
                    nc.scalar.copy(qraw[:], qp[:])
                    sqt = qsb.tile([128, blk], F16, name="sqt", tag="sqt")
                    nc.vector.tensor_mul(sqt[:], qraw[:], qraw[:])
                    nc.tensor.matmul(
                        sq_ps[:],
                        ohq_sb[:, oc, :],
                        sqt[:],
                        start=(oc == 0),
                        stop=(oc == NH - 1),
                    )
                    rp = qps.tile([128, blk], F32, name="rope_ps", tag="rps")
                    nc.tensor.matmul(rp[:], rq_sb[:], qraw[:], start=True, stop=True)
                    nc.scalar.copy(qro[:, oc, :], rp[:])
                sqq = sqqp.tile([128, blk], F32, name="sqq", tag="sqq")
                # A = sumsq_q/128 + eps
                nc.scalar.activation(
                    sqq[:], sq_ps[:], AF.Identity, bias=qbi_sb,
                    scale=1.0 / HD,
                )
                qrot_b.append(qro)
                sqq_b.append(sqq)

        # ---------------- Phase KV + attention ----------------
        with (
            tc.tile_pool(name="wkvp", bufs=1) as wkvp,
            tc.tile_pool(name="kvtp", bufs=2) as kvtp,
            tc.tile_pool(name="kvps", bufs=2, space="PSUM") as kvps,
            tc.tile_pool(name="ksb", bufs=2) as ksb,
            tc.tile_pool(name="asb", bufs=1) as asb,
            tc.tile_pool(name="yp", bufs=1) as yp,
        ):
            wk_sb = wkvp.tile([128, DC, 512], F16, name="wk_sb")
            nc.sync.dma_start(wk_sb[:], wk_d[:])
            wv_sb = wkvp.tile([128, DC, 512], F16, name="wv_sb")
            nc.sync.dma_start(wv_sb[:], wv_d[:])

            for b in range(nblk):
                qro = qrot_b[b]
                sc_ps = kvps.tile([128, blk], F32, name="sc_ps", tag="sc", bufs=1)
                sqk_ps = kvps.tile([128, blk], F32, name="sqk_ps", tag="sqk", bufs=1)
                # ---- pass 1: K path + scores ----
                for dep in range(DEP):
                    kvt = kvtp.tile([128, DC, blk], F16, name="kvt_t", tag="kvt")
                    nc.sync.dma_start(
                        kvt[:], kvt_d[dep, :, :, b * blk : (b + 1) * blk]
                    )
                    for kc in range(NKV):
                        kps = kvps.tile([128, blk], F32, name="k_ps", tag="kps")
                        for c in range(DC):
                            nc.tensor.matmul(
                                kps[:],
                                wk_sb[:, c, kc * 128 : (kc + 1) * 128],
                                kvt[:, c, :],
                                start=(c == 0),
                                stop=(c == DC - 1),
                            )
                        kraw = ksb.tile([128, blk], F16, name="kraw", tag="kraw")
                        nc.scalar.copy(kraw[:], kps[:])
                        ksq = ksb.tile([128, blk], F16, name="ksq", tag="ksq")
                        nc.vector.tensor_mul(ksq[:], kraw[:], kraw[:])
                        nc.tensor.matmul(
                            sqk_ps[:],
                            ohk_sb[:, kc * DEP + dep, :],
                            ksq[:],
                            start=(dep == 0 and kc == 0),
                            stop=(dep == DEP - 1 and kc == NKV - 1),
                        )
                        krp = kvps.tile([128, blk], F32, name="kr_ps", tag="krp")
                        nc.tensor.matmul(
                            krp[:], rk_sb[:, dep, :], kraw[:], start=True, stop=True
                        )
                        krot = ksb.tile(
                            [128, blk], F16, name="krot", tag=f"krot{kc}"
                        )
                        nc.scalar.copy(krot[:], krp[:])
                        for hh in range(4):
                            h = kc * 4 + hh
                            cmb = h * 8 + dep
                            prod = ksb.tile(
                                [128, blk], F16, name="prod", tag="prod", bufs=3
                            )
                            nc.vector.tensor_mul(prod[:], qro[:, h, :], krot[:])
                            nc.tensor.matmul(
                                sc_ps[:],
                                ohsc_sb[:, 128 - cmb : 256 - cmb],
                                prod[:],
                                start=(dep == 0 and kc == 0 and hh == 0),
                                stop=(dep == DEP - 1 and kc == NKV - 1 and hh == 3),
                            )

                # ---- softmax / attn for this block ----
                sqk = asb.tile([128, blk], F32, name="sqk", tag="t1")
                nc.scalar.activation(
                    sqk[:], sqk_ps[:], AF.Identity, bias=kbi_sb,
                    scale=ksc_sb,
                )
                den = asb.tile([128, blk], F32, name="den", tag="t2")
                nc.vector.tensor_mul(den[:], sqq_b[b][:], sqk[:])
                dens = asb.tile([128, blk], F32, name="dens", tag="t1")
                nc.scalar.sqrt(dens[:], den[:])
                rec = asb.tile([128, blk], F32, name="rec", tag="t2")
                nc.vector.reciprocal(rec[:], dens[:])
                ssc = asb.tile([128, blk], F32, name="ssc", tag="t1")
                nc.vector.tensor_mul(ssc[:], sc_ps[:], rec[:])
                epx = asb.tile([128, blk], F16, name="epx", tag="e1")
                nc.scalar.activation(epx[:], ssc[:], AF.Exp, bias=ebi_sb)
                sxp_ps = kvps.tile([128, blk], F32, name="sxp_ps", tag="vps")
                nc.tensor.matmul(sxp_ps[:], bd_sb[:], epx[:], start=True, stop=True)
                rec2 = asb.tile([128, blk], F32, name="rec2", tag="t1")
                nc.vector.reciprocal(rec2[:], sxp_ps[:])
                attn = asb.tile([128, blk], F16, name="attn", tag="e2")
                nc.vector.tensor_mul(attn[:], epx[:], rec2[:])
                attnU = asb.tile([128, nsubs, 128], F32, name="attnU", tag="attnU")
                for ns in range(nsubs):
                    atp = kvps.tile([128, 128], F16, name="at_ps", tag="kps")
                    nc.tensor.transpose(
                        atp[:], attn[:, ns * 128 : (ns + 1) * 128], id_sb[:]
                    )
                    nc.scalar.copy(attnU[:, ns, :], atp[:])

                # ---- pass 2: V path + y accumulation ----
                y_acc = yp.tile([128, nsubs, DIM], F16, name="y_acc", tag="yacc")
                for dep in range(DEP):
                    kvt2 = kvtp.tile([128, DC, blk], F16, name="kvt2", tag="kvt")
                    nc.sync.dma_start(
                        kvt2[:], kvt_d[dep, :, :, b * blk : (b + 1) * blk]
                    )
                    for ns in range(nsubs):
                        vps_t = kvps.tile([128, 512], F32, name="v_ps", tag="vps")
                        for c in range(DC):
                            nc.tensor.matmul(
                                vps_t[:],
                                kvt2[:, c, ns * 128 : (ns + 1) * 128],
                                wv_sb[:, c, :],
                                start=(c == 0),
                                stop=(c == DC - 1),
                            )
                        v_sb = ksb.tile([128, 512], F16, name="v_sb", tag="v")
                        nc.scalar.copy(v_sb[:], vps_t[:])
                        if dep == 0:
                            for h in range(NH):
                                g = h // 4
                                nc.vector.tensor_scalar_mul(
                                    y_acc[:, ns, h * 128 : (h + 1) * 128],
                                    v_sb[:, g * 128 : (g + 1) * 128],
                                    attnU[:, ns, h * 8 + dep : h * 8 + dep + 1],
                                )
                        else:
                            tmp = yp.tile([128, DIM], F16, name="tmp", tag="tmp",
                                          bufs=2)
                            for h in range(NH):
                                g = h // 4
                                nc.vector.tensor_scalar_mul(
                                    tmp[:, h * 128 : (h + 1) * 128],
                                    v_sb[:, g * 128 : (g + 1) * 128],
                                    attnU[:, ns, h * 8 + dep : h * 8 + dep + 1],
                                )
                            nc.vector.tensor_add(
                                y_acc[:, ns, :], y_acc[:, ns, :], tmp[:]
                            )
                # ---- transpose y -> yT, spill to DRAM ----
                for ns in range(nsubs):
                    yts = yp.tile([128, DC, 128], F16, name="yts", tag="yts",
                                  bufs=2)
                    for fc in range(DC):
                        ytps = kvps.tile([128, 128], F16, name="yt_ps", tag="krp")
                        nc.tensor.transpose(
                            ytps[:],
                            y_acc[:, ns, fc * 128 : (fc + 1) * 128],
                            id_sb[:],
                        )
                        nc.scalar.copy(yts[:, fc, :], ytps[:])
                    nc.sync.dma_start(yt_d[b, ns], yts[:])

        # ---------------- Phase O ----------------
        with (
            tc.tile_pool(name="wpp", bufs=1) as wpp,
            tc.tile_pool(name="ytrp", bufs=3) as ytrp,
            tc.tile_pool(name="ops", bufs=2, space="PSUM") as opsp,
            tc.tile_pool(name="osb", bufs=3) as osbp,
        ):
            wp_sb = wpp.tile([128, DC, DIM], F16, name="wp_sb")
            nc.sync.dma_start(wp_sb[:], wp_d[:])
            for b in range(nblk):
                for ns in range(nsubs):
                    yt = ytrp.tile([128, DC, 128], F16, name="yt_r", tag="ytr")
                    nc.sync.dma_start(yt[:], yt_d[b, ns])
                    ops_t = [
                        opsp.tile([128, 512], F32, name=f"o_ps{og}", tag=f"og{og}")
                        for og in range(4)
                    ]
                    for fc in range(DC):
                        for og in range(4):
                            nc.tensor.matmul(
                                ops_t[og][:],
                                yt[:, fc, :],
                                wp_sb[:, fc, og * 512 : (og + 1) * 512],
                                start=(fc == 0),
                                stop=(fc == DC - 1),
                            )
                    row = b * blk + ns * 128
                    for og in range(4):
                        ost = osbp.tile([128, 512], F32, name="ost", tag="ost")
                        nc.scalar.copy(ost[:], ops_t[og][:])
                        nc.sync.dma_start(
                            out_d[row : row + 128, og * 512 : (og + 1) * 512],
                            ost[:],
                        )
    nc.finalize()
    return nc


# ---------------------------------------------------------------- pjrt runner
class _Runner:
    """Persistent jitted shard_map executor (no output donation, so the
    compiled callable can be re-invoked for timing)."""

    def __init__(self, nc, n_cores):
        import jax
        from jax.experimental.shard_map import shard_map
        from jax.sharding import Mesh, NamedSharding, PartitionSpec

        from concourse import bass2jax

        bass2jax.install_neuronx_cc_hook()
        self.jax = jax
        self.nc = nc
        self.n_cores = n_cores

        in_names, out_names, out_avals = [], [], []
        partition_name = (
            nc.partition_id_tensor.name if nc.partition_id_tensor else None
        )
        for alloc in nc.m.functions[0].allocations:
            if not isinstance(alloc, mybir.MemoryLocationSet):
                continue
            name = alloc.memorylocations[0].name
            if alloc.kind == "ExternalInput":
                if name != partition_name:
                    in_names.append(name)
            elif alloc.kind == "ExternalOutput":
                out_names.append(name)
                shape = tuple(alloc.tensor_shape)
                dtype = mybir.dt.np(alloc.dtype)
                out_avals.append(jax.core.ShapedArray(shape, dtype))
        self.param_names = list(in_names)
        self.out_names = list(out_names)
        self.out_avals = out_avals
        all_in_names = in_names + out_names
        if partition_name is not None:
            all_in_names.append(partition_name)

        def _body(*args):
            operands = list(args)
            if partition_name is not None:
                operands.append(bass2jax.partition_id_tensor())
            outs = bass2jax._bass_exec_p.bind(
                *operands,
                out_avals=tuple(out_avals),
                in_names=tuple(all_in_names),
                out_names=tuple(out_names),
                lowering_input_output_aliases=(),
                sim_require_finite=True,
                sim_require_nnan=True,
                nc=nc,
            )
            return tuple(outs)

        devices = jax.devices()[:n_cores]
        assert len(devices) == n_cores
        self.mesh = Mesh(np.asarray(devices), ("core",))
        spec = PartitionSpec("core")
        n_all = len(self.param_names) + len(out_names)
        self.sharding = NamedSharding(self.mesh, spec)
        self.fn = jax.jit(
            shard_map(
                _body,
                mesh=self.mesh,
                in_specs=(spec,) * n_all,
                out_specs=(spec,) * len(out_names),
                check_rep=False,
            ),
            keep_unused=True,
        )
        self.dev_args = None

    def put(self, in_maps):
        jax = self.jax
        concat = [
            np.concatenate([np.asarray(m[name]) for m in in_maps], axis=0)
            for name in self.param_names
        ]
        zeros = [
            np.zeros((self.n_cores * a.shape[0], *a.shape[1:]), a.dtype)
            for a in self.out_avals
        ]
        self.dev_args = [
            jax.device_put(a, self.sharding) for a in (concat + zeros)
        ]
        jax.block_until_ready(self.dev_args)

    def run(self):
        outs = self.fn(*self.dev_args)
        self.jax.block_until_ready(outs)
        return outs

    def time_exec(self, iters=20):
        import time as _t

        self.run()  # warm
        times = []
        for _ in range(iters):
            t0 = _t.perf_counter()
            self.run()
            times.append(_t.perf_counter() - t0)
        return times


_RUNNER = None


# ---------------------------------------------------------------- entry point
def kernel(x, depth_history, Wq, Wk, Wv, Wproj, q_gain):
    global _RUNNER
    x = np.asarray(x, np.float32)
    depth_history = np.asarray(depth_history, np.float32)
    B, S, D = x.shape
    N = B * S
    npc = N // NCORES

    consts = _host_constants(np.asarray(q_gain, np.float32))
    weights = _prep_weights(
        np.asarray(Wq, np.float32),
        np.asarray(Wk, np.float32),
        np.asarray(Wv, np.float32),
        np.asarray(Wproj, np.float32),
    )
    kvt = _prep_kvt(x, depth_history)  # [8, 128, 16, N] strided fp16

    key = npc
    if key not in _PROGRAM_CACHE:
        _PROGRAM_CACHE[key] = build_program(npc)
    nc = _PROGRAM_CACHE[key]

    shared = dict(weights)
    shared.update(consts)
    in_maps = []
    for core in range(NCORES):
        m = dict(shared)
        m["kvt"] = np.ascontiguousarray(
            kvt[:, :, :, core * npc : (core + 1) * npc]
        )
        in_maps.append(m)

    if _RUNNER is None or _RUNNER.nc is not nc:
        _RUNNER = _Runner(nc, NCORES)
    _RUNNER.put(in_maps)
    outs = _RUNNER.run()
    oidx = _RUNNER.out_names.index("out")
    out = np.asarray(outs[oidx])
    return out.reshape(B, S, D).astype(np.float32)
